# revision 2
# baseline (speedup 1.0000x reference)
"""Trainium2 Bass kernel for nn_CIFARDiffusionLayer.

The reference applies, per channel c, three ADI steps; each step is an
x-sweep (constant-coefficient tridiagonal solve along W), a y-sweep
(same along H), and a multiply by diag(channel_coupling)[c].  Every
sweep is a fixed linear map, so the whole layer collapses to

    out[b, c] = s_c^3 * (A_c @ u[b, c] @ B_c)      (s_c = coupling diag)

with A_c, B_c tiny 256x256 matrices computed on the host in float64
from the reference's exact recurrences (including its eps quirks).

This version is tuned for the memory roofline (360 GB/s DMA bus/core):

* fp16 wire format.  The correctness gate is 2e-2; fp16 end-to-end
  costs ~4e-4.  The host converts u to fp16 (pre-swizzled into the
  device layout), the device computes fp16 matmuls (f32 PSUM), writes
  fp16, and the host upcasts + applies the s_c^3 scale during the
  gather.  Per-core HBM traffic drops from ~26 MB to ~13 MB.
* Banded matmuls.  A_c/B_c entries decay by ~7.5e-3 per off-diagonal,
  so both are banded to +/-4 (dropped mass ~8e-10).  With the data
  slab stationary and the matrices moving, each 256-contraction is
  split into per-128-chunk column segments: [0,124) and [132,256) are
  single-pass, only the 8-column boundary region [124,132) accumulates
  over both chunks.  Moving rows per slab drop 2048 -> 1056.
* Weight-load elision.  Consecutive matmuls that reuse the same
  stationary data quad set InstMatmult.ldweights=False, halving the
  LDWEIGHTS column traffic on the PE weight port (fp16 also enables
  the compiler's fast-weight-load path, unlike the f32r baseline).

Sharding: data parallelism over (batch, channel) slabs: 384 slabs are
dealt to 8 cores as 48 slabs each (32 of one channel + 16 of another,
per the ASSIGN table), so each core loads only the 2 matrix pairs it
needs while the NEFF stays identical across cores.
"""

import sys

if "/opt/trn_rl_repo" not in sys.path:
    sys.path.insert(0, "/opt/trn_rl_repo")

import numpy as np

DT = 0.05
DX = 1.0
NUM_STEPS = 3
EPS = 1e-6
MAX_COEFF = 1.0

N_CORES = 8
B, C, S = 128, 3, 256
N_SLAB = 48          # (batch, channel) slabs per core
N_GROUP = N_SLAB // 3
OV0, OV1 = 124, 132  # PSUM columns accumulating over both 128-chunks (band +/-4)
# Per core: ((channel of the 32-slab block, batch start), (channel of the
# 16-slab block, batch start)).  Covers each (b, c) exactly once:
# c0 = 4x32, c1 = 2x32 + 4x16, c2 = 2x32 + 4x16.
ASSIGN = [
    ((0, 0), (1, 64)),
    ((0, 32), (1, 80)),
    ((0, 64), (1, 96)),
    ((0, 96), (1, 112)),
    ((1, 0), (2, 64)),
    ((1, 32), (2, 80)),
    ((2, 0), (2, 96)),
    ((2, 32), (2, 112)),
]


def _core_slab_indices(k):
    (c32, b32), (c16, b16) = ASSIGN[k]
    b_idx = list(range(b32, b32 + 32)) + list(range(b16, b16 + 16))
    c_idx = [c32] * 32 + [c16] * 16
    return b_idx, c_idx


def _thomas_inv(r: float, n: int = S, eps: float = EPS) -> np.ndarray:
    """T^{-1} for the reference's constant-coefficient Thomas solve.

    Mirrors reference._thomas_const exactly (b[0]+eps on the first
    denominator, clamp(min=eps) on interior denominators), evaluated in
    float64 on the identity RHS so columns are T^{-1} e_j.
    """
    a = -r
    b = np.full(n, 1.0 + 2.0 * r, dtype=np.float64)
    b[0] = b[-1] = 1.0 + r
    denom = np.empty(n, dtype=np.float64)
    cp = np.empty(n, dtype=np.float64)
    denom[0] = b[0] + eps
    cp[0] = a / denom[0]
    for i in range(1, n):
        denom[i] = max(b[i] - a * cp[i - 1], eps)
        cp[i] = a / denom[i]
    dp = np.zeros((n, n), dtype=np.float64)
    eye = np.eye(n, dtype=np.float64)
    dp[0] = eye[0] / denom[0]
    for i in range(1, n):
        dp[i] = (eye[i] - a * dp[i - 1]) / denom[i]
    x = np.zeros((n, n), dtype=np.float64)
    x[n - 1] = dp[n - 1]
    for i in range(n - 2, -1, -1):
        x[i] = dp[i] - cp[i] * x[i + 1]
    return x


def _dev_layout(m: np.ndarray) -> np.ndarray:
    """[256, 256] -> [128, 512] fp16 SBUF image: tile[p, k*256+j] = m[k*128+p, j]."""
    return np.ascontiguousarray(
        m.reshape(2, 128, 256).transpose(1, 0, 2).reshape(128, 512).astype(np.float16)
    )


def _host_mats(alpha_base, beta_base, alpha_spatial, beta_spatial, channel_coupling):
    """Device-layout mats [C, 2, 128, 512] fp16 (s^3 NOT folded) + s3 [C] f64.

    mats[c, 0] holds A_c^T (so rhs[p=h, j=h'] = A[h', h]), mats[c, 1]
    holds B_c (rhs[p=w, j=w'] = B[w, w']).
    """
    diag = np.diagonal(np.asarray(channel_coupling)).astype(np.float64)
    mats = np.empty((C, 2, 128, 512), dtype=np.float16)
    for c in range(C):
        am = float(np.mean(np.asarray(alpha_spatial[c], dtype=np.float64)))
        bm = float(np.mean(np.asarray(beta_spatial[c], dtype=np.float64)))
        a_c = np.eye(S, dtype=np.float64)
        b_c = np.eye(S, dtype=np.float64)
        for step in range(NUM_STEPS):
            t = step * DT
            alpha_t = min(max(float(alpha_base[c]) + am * t, EPS), MAX_COEFF)
            beta_t = min(max(float(beta_base[c]) + bm * t, EPS), MAX_COEFF)
            r_a = alpha_t * (DT / 2.0) / DX**2
            r_b = beta_t * (DT / 2.0) / DX**2
            a_c = _thomas_inv(r_b) @ a_c
            b_c = b_c @ _thomas_inv(r_a).T
        mats[c, 0] = _dev_layout(a_c.T)
        mats[c, 1] = _dev_layout(b_c)
    return mats, diag**3


def prep_inputs(u, alpha_base, beta_base, alpha_spatial, beta_spatial, channel_coupling):
    """Host staging: per-core fp16 device-layout inputs.

    Returns (u_cores [8][48,128,512] fp16, mats_cores [8][2,2,128,512] fp16,
    s3 [C] f64, idxs).
    """
    mats_full, s3 = _host_mats(
        alpha_base, beta_base, alpha_spatial, beta_spatial, channel_coupling
    )
    u16 = np.asarray(u, dtype=np.float16)  # one-pass downcast of the full tensor
    u_cores, mats_cores, idxs = [], [], []
    for k in range(N_CORES):
        b_idx, c_idx = _core_slab_indices(k)
        idxs.append((b_idx, c_idx))
        uc = u16[b_idx, c_idx]  # [48, 256, 256]
        uc = np.ascontiguousarray(
            uc.reshape(N_SLAB, 2, 128, 256).transpose(0, 2, 1, 3).reshape(N_SLAB, 128, 512)
        )
        u_cores.append(uc)
        (c32, _), (c16, _) = ASSIGN[k]
        mats_cores.append(np.stack([mats_full[c32], mats_full[c16]]))
    return u_cores, mats_cores, s3, idxs


def build_module(repeat: int = 1):
    """Per-core Bass module: out[b,c] = A_c @ u[b,c] @ B_c for 48 fp16 slabs.

    repeat > 1 wraps the batch loop in a hardware For_i that re-runs the
    whole kernel body; only used by the timing harness (wall-clock slope
    between two repeat counts isolates the per-iteration device time).
    """
    import concourse.bacc as bacc
    import concourse.tile as tile
    from concourse import mybir

    f16, f32 = mybir.dt.float16, mybir.dt.float32
    nc = bacc.Bacc(
        "TRN2",
        target_bir_lowering=False,
        debug=False,
        enable_asserts=False,
        num_devices=N_CORES,
    )
    u_d = nc.dram_tensor("u", [N_SLAB, 128, 512], f16, kind="ExternalInput")
    m_d = nc.dram_tensor("mats", [2, 2, 128, 512], f16, kind="ExternalInput")
    o_d = nc.dram_tensor("out", [N_GROUP, 128, 3 * 512], f16, kind="ExternalOutput")

    with tile.TileContext(nc) as tc:
        with (
            tc.tile_pool(name="consts", bufs=1) as cpool,
            tc.tile_pool(name="ld", bufs=5) as ldpool,
            tc.tile_pool(name="vt", bufs=3) as vtpool,
            tc.tile_pool(name="zs", bufs=4) as zspool,
            tc.tile_pool(name="pv", bufs=2, space="PSUM") as pvpool,
            tc.tile_pool(name="pz", bufs=2, space="PSUM") as pzpool,
        ):
            # Matrix pair q in {0,1}; [128, 512] fp16 per (pair, side):
            # free = k*256 + j, i.e. contraction chunk k on partitions.
            a_t, b_t = [], []
            for q in range(2):
                at = cpool.tile([128, 512], f16, tag=f"a{q}")
                nc.sync.dma_start(at[:], m_d[q, 0])
                a_t.append(at)
                bt = cpool.tile([128, 512], f16, tag=f"b{q}")
                nc.sync.dma_start(bt[:], m_d[q, 1])
                b_t.append(bt)

            def banded_mm(out_ps, quad, mat, mo):
                """out_ps[:, mo*256+j] = sum_h quad(k)[h, :].T @ mat[k-chunk, j]
                over the +/-4 band: chunks are single-pass except the 8
                boundary columns [OV0, OV1) which accumulate across both.
                Matmuls reusing the in-array stationary quad skip LDWEIGHTS.
                """
                ob = lambda c0, c1: out_ps[:, mo * 256 + c0 : mo * 256 + c1]
                mA = nc.tensor.matmul(
                    ob(0, OV0), quad(0), mat[:, 0:OV0], start=True, stop=True
                )
                mB = nc.tensor.matmul(
                    ob(OV0, OV1), quad(0), mat[:, OV0:OV1], start=True, stop=False
                )
                mB.ins.ldweights = False
                mC = nc.tensor.matmul(
                    ob(OV0, OV1), quad(1), mat[:, 256 + OV0 : 256 + OV1],
                    start=False, stop=True,
                )
                mD = nc.tensor.matmul(
                    ob(OV1, 256), quad(1), mat[:, 256 + OV1 : 512], start=True, stop=True
                )
                mD.ins.ldweights = False

            def batch_loop():
                for g in range(N_GROUP):
                    _emit_group(g)

            def _emit_group(g):
                # Load 3 slabs: free layout j*512 + k*256 + w, partition = h%128.
                # Per-slab DMAs keep the SP queue from head-of-line blocking.
                ld = ldpool.tile([128, 3 * 512], f16)
                for j in range(3):
                    nc.sync.dma_start(ld[:, j * 512 : (j + 1) * 512], u_d[3 * g + j])
                zs = zspool.tile([128, 3 * 512], f16)
                for j in range(3):
                    slab = 3 * g + j
                    q = 0 if slab < 32 else 1
                    base = j * 512
                    # MM1: V^T[w, h'] = sum_h U[h, w] * A^T[h, h']  (data stationary)
                    pv = pvpool.tile([128, 512], f32)
                    for mi in range(2):
                        quad = lambda k: ld[
                            :, base + k * 256 + mi * 128 : base + k * 256 + mi * 128 + 128
                        ]
                        banded_mm(pv, quad, a_t[q], mi)
                    vt = vtpool.tile([128, 512], f16)
                    nc.vector.tensor_copy(vt[:], pv[:])
                    # MM2: Z[h', w'] = sum_w V^T[w, h'] * B[w, w']
                    pz = pzpool.tile([128, 512], f32)
                    for mo in range(2):
                        quad = lambda k: vt[
                            :, k * 256 + mo * 128 : k * 256 + mo * 128 + 128
                        ]
                        banded_mm(pz, quad, b_t[q], mo)
                    nc.scalar.copy(zs[:, base : base + 512], pz[:])
                # Out-DMA on the ACT HWDGE ring: keeps the SP queue free for
                # input loads (out-DMAs wait on compute; SP head-of-line
                # blocking would stall the next group's loads behind them).
                nc.scalar.dma_start(o_d[g], zs[:])

            if repeat == 1:
                batch_loop()
            else:
                # staggered_reset avoids the ~3us all-engine barrier at the
                # loop back-edge, so the slope measurement better matches the
                # barrier-free single-shot kernel.
                with tc.For_i(0, repeat, 1, staggered_reset=True):
                    batch_loop()
    nc.compile()
    return nc


_CACHE = {}


def _axon_runner():
    """Build (once) a jitted 8-way sharded executor for the axon/PJRT path.

    Mirrors concourse.bass2jax.run_bass_via_pjrt but keeps the compiled
    executable alive so repeat kernel() calls skip retracing + NEFF
    recompilation.
    """
    if "runner" in _CACHE:
        return _CACHE["runner"]
    import jax
    from jax.experimental.shard_map import shard_map
    from jax.sharding import Mesh, NamedSharding, PartitionSpec

    from concourse import bass2jax, mybir

    nc = build_module()
    bass2jax.install_neuronx_cc_hook()
    partition_name = nc.partition_id_tensor.name if nc.partition_id_tensor else None
    in_names, out_names, out_avals = [], [], []
    for alloc in nc.m.functions[0].allocations:
        if not isinstance(alloc, mybir.MemoryLocationSet):
            continue
        name = alloc.memorylocations[0].name
        if alloc.kind == "ExternalInput":
            if name != partition_name:
                in_names.append(name)
        elif alloc.kind == "ExternalOutput":
            out_names.append(name)
            out_avals.append(
                jax.core.ShapedArray(tuple(alloc.tensor_shape), mybir.dt.np(alloc.dtype))
            )
    n_params = len(in_names)
    n_outs = len(out_avals)
    all_names = in_names + out_names + ([partition_name] if partition_name else [])
    donate = tuple(range(n_params, n_params + n_outs))

    def _body(*args):
        operands = list(args)
        if partition_name is not None:
            operands.append(bass2jax.partition_id_tensor())
        return tuple(
            bass2jax._bass_exec_p.bind(
                *operands,
                out_avals=tuple(out_avals),
                in_names=tuple(all_names),
                out_names=tuple(out_names),
                lowering_input_output_aliases=(),
                sim_require_finite=True,
                sim_require_nnan=True,
                nc=nc,
            )
        )

    devices = jax.devices()[:N_CORES]
    mesh = Mesh(np.asarray(devices), ("core",))
    spec = NamedSharding(mesh, PartitionSpec("core"))
    sharded = jax.jit(
        shard_map(
            _body,
            mesh=mesh,
            in_specs=(PartitionSpec("core"),) * (n_params + n_outs),
            out_specs=(PartitionSpec("core"),) * n_outs,
            check_rep=False,
        ),
        donate_argnums=donate,
        keep_unused=True,
    )

    def run(u_cores, mats_cores):
        per_core = {
            "u": np.concatenate(u_cores, axis=0),
            "mats": np.concatenate(mats_cores, axis=0),
        }
        xs = [jax.device_put(per_core[nm], spec) for nm in in_names]
        zs = [
            jax.device_put(
                np.zeros((N_CORES * a.shape[0], *a.shape[1:]), a.dtype), spec
            )
            for a in out_avals
        ]
        outs = sharded(*xs, *zs)
        out = np.asarray(outs[out_names.index("out")])
        return out.reshape(N_CORES, N_GROUP, 128, 3 * 512)

    _CACHE["runner"] = run
    return run


def kernel(u, alpha_base, beta_base, alpha_spatial, beta_spatial, channel_coupling):
    from concourse._compat import axon_active

    u_cores, mats_cores, s3, idxs = prep_inputs(
        np.asarray(u, dtype=np.float32),
        np.asarray(alpha_base, dtype=np.float32),
        np.asarray(beta_base, dtype=np.float32),
        np.asarray(alpha_spatial, dtype=np.float32),
        np.asarray(beta_spatial, dtype=np.float32),
        np.asarray(channel_coupling, dtype=np.float32),
    )

    if axon_active():
        res = _axon_runner()(u_cores, mats_cores)
    else:
        # Native path (/dev/neuron* present): run via NRT on cores 0-7.
        from concourse.bass_utils import run_bass_kernel_spmd

        nc = _CACHE.setdefault("nc", build_module())
        in_maps = [
            {"u": u_cores[k], "mats": mats_cores[k]} for k in range(N_CORES)
        ]
        rr = run_bass_kernel_spmd(nc, in_maps, core_ids=list(range(N_CORES)))
        res = np.stack([r["out"] for r in rr.results])

    out = np.empty((B, C, S, S), dtype=np.float32)
    for k in range(N_CORES):
        b_idx, c_idx = idxs[k]
        # res[k]: [16, 128, 3*512] with free = j*512 + mo*256 + w, part = h'%128
        core = (
            res[k]
            .reshape(N_GROUP, 128, 3, 2, 256)
            .transpose(0, 2, 3, 1, 4)
            .reshape(N_SLAB, S, S)
            .astype(np.float32)
        )
        core *= s3[c_idx][:, None, None]
        out[b_idx, c_idx] = core
    return out
